# revision 17
# baseline (speedup 1.0000x reference)
"""Trainium2 Bass kernel for nn_AttBlock (demographic attention gating).

y[n,c,h,w] = x[n,c,h,w] * sigmoid(att_channel)[d_n,c] * up2(sigmoid(att_spatial))[d_n,h,w]
with d_n = demog_label[n]. Also returns ac = sigmoid(att_channel) and
asp = nearest-up2(sigmoid(att_spatial)).

Strategy: data-parallel over batch N across 8 NeuronCores (8 samples each).
On-device label gather via one-hot matmul; per-sample combined gate built as
a PE outer-product (gate_c ⊗ gate_s) into PSUM; single DVE multiply per chunk.
DMA-bound: 25.7MB in + 25.7MB out per core.
"""

import numpy as np

N, C, H, W = 64, 256, 56, 56
NDEMOG = 4
HS, WS = 28, 28
HWF = H * W  # 3136
SPF = HS * WS  # 784
N_CORES = 8
N_LOC = N // N_CORES  # 8 samples per core
CHUNK = 448
NCHUNK = HWF // CHUNK  # 7

_BUILD_CACHE = {}


def _build():
    if "nc" in _BUILD_CACHE:
        return _BUILD_CACHE["nc"]

    import concourse.bacc as bacc
    import concourse.mybir as mybir
    import concourse.tile as tile

    f32 = mybir.dt.float32
    bf16 = mybir.dt.bfloat16
    i32 = mybir.dt.int32
    AF = mybir.ActivationFunctionType
    ALU = mybir.AluOpType

    nc = bacc.Bacc(None, target_bir_lowering=False)

    x_ext = nc.declare_dram_parameter("x", [N_LOC, C, HWF], f32, isOutput=False)
    lab_ext = nc.declare_dram_parameter("labels", [N_LOC], i32, isOutput=False)
    ac_ext = nc.declare_dram_parameter("att_channel", [NDEMOG, C], f32, isOutput=False)
    asp_ext = nc.declare_dram_parameter("att_spatial", [NDEMOG, SPF], f32, isOutput=False)
    y_ext = nc.declare_dram_parameter("y", [N_LOC, C, HWF], bf16, isOutput=True)
    oac_ext = nc.declare_dram_parameter("ac", [NDEMOG, C], f32, isOutput=True)
    oasp_ext = nc.declare_dram_parameter("asp", [NDEMOG, HWF], f32, isOutput=True)

    with tile.TileContext(nc) as tc:
        with (
            tc.tile_pool(name="const", bufs=1) as cpool,
            tc.tile_pool(name="xy", bufs=8) as xpool,
            tc.tile_pool(name="yy", bufs=4) as ypool,
            tc.tile_pool(name="psg", bufs=3, space="PSUM") as gpool,
        ):
            # ---- channel gate table: ac = sigmoid(att_channel), also an output ----
            ac_raw = cpool.tile([NDEMOG, C], f32)
            nc.sync.dma_start(ac_raw[:], ac_ext[:])
            ac_sig = cpool.tile([NDEMOG, C], f32)
            nc.scalar.activation(ac_sig[:], ac_raw[:], AF.Sigmoid)
            nc.sync.dma_start(oac_ext[:], ac_sig[:])

            # ---- spatial gate table: sigmoid then nearest-2x upsample (28->56) ----
            sp_raw = cpool.tile([NDEMOG, SPF], f32)
            nc.sync.dma_start(sp_raw[:], asp_ext[:])
            sp_sig = cpool.tile([NDEMOG, SPF], f32)
            nc.scalar.activation(sp_sig[:], sp_raw[:], AF.Sigmoid)
            # nearest-2x upsample via strided DVE copies (no tiny-descriptor DMAs)
            sp_wide = cpool.tile([NDEMOG, HS, W], f32)  # columns doubled
            for s in range(2):
                nc.vector.tensor_copy(
                    sp_wide[:, :, s::2], sp_sig[:].rearrange("d (h w) -> d h w", h=HS)
                )
            sp_up = cpool.tile([NDEMOG, HWF], f32)
            sp_up_v = sp_up[:].rearrange("d (h r f) -> d h r f", h=HS, r=2)
            for r in range(2):
                nc.vector.tensor_copy(sp_up_v[:, :, r, :], sp_wide[:])
            nc.sync.dma_start(oasp_ext[:], sp_up[:])

            # ---- one-hot of labels: onehot[d, n] = (labels[n] == d) ----
            lab_i = cpool.tile([NDEMOG, N_LOC], i32)
            nc.sync.dma_start(lab_i[:], lab_ext[:][None, :].partition_broadcast(NDEMOG))
            lab_f = cpool.tile([NDEMOG, N_LOC], f32)
            nc.vector.tensor_copy(lab_f[:], lab_i[:])
            iota_i = cpool.tile([NDEMOG, 1], i32)
            nc.gpsimd.iota(iota_i[:], pattern=[[0, 1]], base=0, channel_multiplier=1)
            iota_f = cpool.tile([NDEMOG, 1], f32)
            nc.vector.tensor_copy(iota_f[:], iota_i[:])
            onehot = cpool.tile([NDEMOG, N_LOC], f32)
            nc.vector.tensor_scalar(
                onehot[:], lab_f[:], iota_f[:], None, op0=ALU.is_equal
            )

            # ---- fold one-hot into the channel table ----
            # acOH[d, n, c] = onehot[d, n] * ac_sig[d, c]; then for sample n the
            # combined gate is a K=4 matmul:
            #   G[m, j] = sum_d acOH[d, n, m] * sp_up[d, j]
            #           = ac_sig[label_n, m] * sp_up[label_n, j]
            # bf16 operands: PE runs 4x faster than fp32, psum accumulation is f32
            acOH = cpool.tile([NDEMOG, N_LOC, C], bf16)
            for n in range(N_LOC):
                nc.vector.tensor_scalar(
                    acOH[:, n, :], ac_sig[:], onehot[:, n : n + 1], None, op0=ALU.mult
                )
            sp_up_bf = cpool.tile([NDEMOG, HWF], bf16)
            nc.vector.tensor_copy(sp_up_bf[:], sp_up[:])

            # ---- main loop: per (sample, channel-half), load x, gate, store y ----
            # loads on the SP HWDGE ring (nc.sync), stores on the ACT ring
            # (nc.scalar) so load/store issue don't serialize on one FIFO.
            for it in range(N_LOC * 2):
                n, half = it // 2, it % 2
                ld_eng = nc.sync if it % 2 == 0 else nc.scalar
                st_eng = nc.scalar if it % 2 == 0 else nc.sync
                xt = xpool.tile([128, HWF], f32, tag="xt")
                yt = ypool.tile([128, HWF], bf16, tag="yt")
                csl = slice(half * 128, (half + 1) * 128)
                ld_eng.dma_start(xt[:], x_ext[n, csl, :])
                lhsT = acOH[:, n, csl]
                # pairs of 448-col chunks share one 2-bank psum tile so the
                # DVE does one multiply per 896 columns
                for k in range(0, NCHUNK - 1, 2):
                    g = gpool.tile([128, 2, 512], f32, tag="g")
                    for j in range(2):
                        ksl = slice((k + j) * CHUNK, (k + j + 1) * CHUNK)
                        nc.tensor.matmul(g[:, j, 0:CHUNK], lhsT, sp_up_bf[:, ksl])
                    pair = slice(k * CHUNK, (k + 2) * CHUNK)
                    xv = xt[:, pair].rearrange("p (two f) -> p two f", two=2)
                    yv = yt[:, pair].rearrange("p (two f) -> p two f", two=2)
                    nc.vector.tensor_mul(yv, xv, g[:, :, 0:CHUNK])
                # odd final chunk
                k = NCHUNK - 1
                g = gpool.tile([128, 2, 512], f32, tag="g")
                ksl = slice(k * CHUNK, (k + 1) * CHUNK)
                nc.tensor.matmul(g[:, 0, 0:CHUNK], lhsT, sp_up_bf[:, ksl])
                nc.vector.tensor_mul(yt[:, ksl], xt[:, ksl], g[:, 0, 0:CHUNK])
                st_eng.dma_start(y_ext[n, csl, :], yt[:])

    nc.compile()
    _BUILD_CACHE["nc"] = nc
    return nc


def kernel(x, demog_label, att_channel, att_spatial):
    from concourse.bass_utils import run_bass_kernel_spmd

    nc = _build()

    x = np.ascontiguousarray(np.asarray(x, dtype=np.float32)).reshape(N, C, HWF)
    labels = np.asarray(demog_label).astype(np.int32)
    acf = np.ascontiguousarray(np.asarray(att_channel, dtype=np.float32)).reshape(
        NDEMOG, C
    )
    aspf = np.ascontiguousarray(np.asarray(att_spatial, dtype=np.float32)).reshape(
        NDEMOG, SPF
    )

    in_maps = []
    for i in range(N_CORES):
        in_maps.append(
            {
                "x": x[i * N_LOC : (i + 1) * N_LOC],
                "labels": labels[i * N_LOC : (i + 1) * N_LOC],
                "att_channel": acf,
                "att_spatial": aspf,
            }
        )

    res = run_bass_kernel_spmd(nc, in_maps, list(range(N_CORES)))

    y = np.concatenate(
        [
            res.results[i]["y"].astype(np.float32).reshape(N_LOC, C, H, W)
            for i in range(N_CORES)
        ],
        axis=0,
    )
    ac = res.results[0]["ac"].reshape(NDEMOG, 1, C, 1, 1)
    asp = res.results[0]["asp"].reshape(NDEMOG, 1, 1, H, W)
    return y, ac, asp


# revision 18
# speedup vs baseline: 1.0474x; 1.0474x over previous
"""Trainium2 Bass kernel for nn_AttBlock (demographic attention gating).

y[n,c,h,w] = x[n,c,h,w] * sigmoid(att_channel)[d_n,c] * up2(sigmoid(att_spatial))[d_n,h,w]
with d_n = demog_label[n]. Also returns ac = sigmoid(att_channel) and
asp = nearest-up2(sigmoid(att_spatial)).

Strategy: data-parallel over batch N across 8 NeuronCores (8 samples each).
On-device label gather via one-hot matmul; per-sample combined gate built as
a PE outer-product (gate_c ⊗ gate_s) into PSUM; single DVE multiply per chunk.
DMA-bound: 25.7MB in + 25.7MB out per core.
"""

import numpy as np

N, C, H, W = 64, 256, 56, 56
NDEMOG = 4
HS, WS = 28, 28
HWF = H * W  # 3136
SPF = HS * WS  # 784
N_CORES = 8
N_LOC = N // N_CORES  # 8 samples per core
CHUNK = 448
NCHUNK = HWF // CHUNK  # 7

_BUILD_CACHE = {}


def _build():
    if "nc" in _BUILD_CACHE:
        return _BUILD_CACHE["nc"]

    import concourse.bacc as bacc
    import concourse.mybir as mybir
    import concourse.tile as tile

    f32 = mybir.dt.float32
    bf16 = mybir.dt.bfloat16
    i32 = mybir.dt.int32
    AF = mybir.ActivationFunctionType
    ALU = mybir.AluOpType

    nc = bacc.Bacc(None, target_bir_lowering=False)

    x_ext = nc.declare_dram_parameter("x", [N_LOC, C, HWF], f32, isOutput=False)
    lab_ext = nc.declare_dram_parameter("labels", [N_LOC], i32, isOutput=False)
    ac_ext = nc.declare_dram_parameter("att_channel", [NDEMOG, C], f32, isOutput=False)
    asp_ext = nc.declare_dram_parameter("att_spatial", [NDEMOG, SPF], f32, isOutput=False)
    y_ext = nc.declare_dram_parameter("y", [N_LOC, C, HWF], bf16, isOutput=True)
    oac_ext = nc.declare_dram_parameter("ac", [NDEMOG, C], f32, isOutput=True)
    oasp_ext = nc.declare_dram_parameter("asp", [NDEMOG, HWF], f32, isOutput=True)

    with tile.TileContext(nc) as tc:
        with (
            tc.tile_pool(name="const", bufs=1) as cpool,
            tc.tile_pool(name="xy", bufs=8) as xpool,
            tc.tile_pool(name="yy", bufs=4) as ypool,
            tc.tile_pool(name="psg", bufs=3, space="PSUM") as gpool,
        ):
            # ---- channel gate table: ac = sigmoid(att_channel), also an output ----
            ac_raw = cpool.tile([NDEMOG, C], f32)
            nc.sync.dma_start(ac_raw[:], ac_ext[:])
            ac_sig = cpool.tile([NDEMOG, C], f32)
            nc.scalar.activation(ac_sig[:], ac_raw[:], AF.Sigmoid)
            nc.sync.dma_start(oac_ext[:], ac_sig[:])

            # ---- spatial gate table: sigmoid then nearest-2x upsample (28->56) ----
            sp_raw = cpool.tile([NDEMOG, SPF], f32)
            nc.sync.dma_start(sp_raw[:], asp_ext[:])
            sp_sig = cpool.tile([NDEMOG, SPF], f32)
            nc.scalar.activation(sp_sig[:], sp_raw[:], AF.Sigmoid)
            # nearest-2x upsample via strided DVE copies (no tiny-descriptor DMAs)
            sp_wide = cpool.tile([NDEMOG, HS, W], f32)  # columns doubled
            for s in range(2):
                nc.vector.tensor_copy(
                    sp_wide[:, :, s::2], sp_sig[:].rearrange("d (h w) -> d h w", h=HS)
                )
            sp_up = cpool.tile([NDEMOG, HWF], f32)
            sp_up_v = sp_up[:].rearrange("d (h r f) -> d h r f", h=HS, r=2)
            for r in range(2):
                nc.vector.tensor_copy(sp_up_v[:, :, r, :], sp_wide[:])
            nc.sync.dma_start(oasp_ext[:], sp_up[:])

            # ---- one-hot of labels: onehot[d, n] = (labels[n] == d) ----
            lab_i = cpool.tile([NDEMOG, N_LOC], i32)
            nc.sync.dma_start(lab_i[:], lab_ext[:][None, :].partition_broadcast(NDEMOG))
            lab_f = cpool.tile([NDEMOG, N_LOC], f32)
            nc.vector.tensor_copy(lab_f[:], lab_i[:])
            iota_i = cpool.tile([NDEMOG, 1], i32)
            nc.gpsimd.iota(iota_i[:], pattern=[[0, 1]], base=0, channel_multiplier=1)
            iota_f = cpool.tile([NDEMOG, 1], f32)
            nc.vector.tensor_copy(iota_f[:], iota_i[:])
            onehot = cpool.tile([NDEMOG, N_LOC], f32)
            nc.vector.tensor_scalar(
                onehot[:], lab_f[:], iota_f[:], None, op0=ALU.is_equal
            )

            # ---- fold one-hot into the channel table ----
            # acOH[d, n, c] = onehot[d, n] * ac_sig[d, c]; then for sample n the
            # combined gate is a K=4 matmul:
            #   G[m, j] = sum_d acOH[d, n, m] * sp_up[d, j]
            #           = ac_sig[label_n, m] * sp_up[label_n, j]
            # bf16 operands: PE runs 4x faster than fp32, psum accumulation is f32
            acOH = cpool.tile([NDEMOG, N_LOC, C], bf16)
            for n in range(N_LOC):
                nc.vector.tensor_scalar(
                    acOH[:, n, :], ac_sig[:], onehot[:, n : n + 1], None, op0=ALU.mult
                )
            sp_up_bf = cpool.tile([NDEMOG, HWF], bf16)
            nc.vector.tensor_copy(sp_up_bf[:], sp_up[:])

            # ---- main loop: per (sample, channel-half), load x, gate, store y ----
            # loads on the SP HWDGE ring (nc.sync), stores on the ACT ring
            # (nc.scalar) so load/store issue don't serialize on one FIFO.
            for it in range(N_LOC * 2):
                n, half = it // 2, it % 2
                ld_eng = nc.sync
                st_eng = nc.scalar
                xt = xpool.tile([128, HWF], f32, tag="xt")
                yt = ypool.tile([128, HWF], bf16, tag="yt")
                csl = slice(half * 128, (half + 1) * 128)
                ld_eng.dma_start(xt[:], x_ext[n, csl, :])
                lhsT = acOH[:, n, csl]
                # pairs of 448-col chunks share one 2-bank psum tile so the
                # DVE does one multiply per 896 columns
                for k in range(0, NCHUNK - 1, 2):
                    g = gpool.tile([128, 2, 512], f32, tag="g")
                    for j in range(2):
                        ksl = slice((k + j) * CHUNK, (k + j + 1) * CHUNK)
                        nc.tensor.matmul(g[:, j, 0:CHUNK], lhsT, sp_up_bf[:, ksl])
                    pair = slice(k * CHUNK, (k + 2) * CHUNK)
                    xv = xt[:, pair].rearrange("p (two f) -> p two f", two=2)
                    yv = yt[:, pair].rearrange("p (two f) -> p two f", two=2)
                    nc.vector.tensor_mul(yv, xv, g[:, :, 0:CHUNK])
                # odd final chunk
                k = NCHUNK - 1
                g = gpool.tile([128, 2, 512], f32, tag="g")
                ksl = slice(k * CHUNK, (k + 1) * CHUNK)
                nc.tensor.matmul(g[:, 0, 0:CHUNK], lhsT, sp_up_bf[:, ksl])
                nc.vector.tensor_mul(yt[:, ksl], xt[:, ksl], g[:, 0, 0:CHUNK])
                st_eng.dma_start(y_ext[n, csl, :], yt[:])

    nc.compile()
    _BUILD_CACHE["nc"] = nc
    return nc


def kernel(x, demog_label, att_channel, att_spatial):
    from concourse.bass_utils import run_bass_kernel_spmd

    nc = _build()

    x = np.ascontiguousarray(np.asarray(x, dtype=np.float32)).reshape(N, C, HWF)
    labels = np.asarray(demog_label).astype(np.int32)
    acf = np.ascontiguousarray(np.asarray(att_channel, dtype=np.float32)).reshape(
        NDEMOG, C
    )
    aspf = np.ascontiguousarray(np.asarray(att_spatial, dtype=np.float32)).reshape(
        NDEMOG, SPF
    )

    in_maps = []
    for i in range(N_CORES):
        in_maps.append(
            {
                "x": x[i * N_LOC : (i + 1) * N_LOC],
                "labels": labels[i * N_LOC : (i + 1) * N_LOC],
                "att_channel": acf,
                "att_spatial": aspf,
            }
        )

    res = run_bass_kernel_spmd(nc, in_maps, list(range(N_CORES)))

    y = np.concatenate(
        [
            res.results[i]["y"].astype(np.float32).reshape(N_LOC, C, H, W)
            for i in range(N_CORES)
        ],
        axis=0,
    )
    ac = res.results[0]["ac"].reshape(NDEMOG, 1, C, 1, 1)
    asp = res.results[0]["asp"].reshape(NDEMOG, 1, 1, H, W)
    return y, ac, asp
